# revision 20
# baseline (speedup 1.0000x reference)
"""Trainium2 Bass kernel for nn_DendriticLayerSiLU_Template.

out = silu(g) * (x @ W.T), where per (token n, unit h):
  a[n,h,w] = sum_s x[n, w*64+s] * T[h, w*64+s]      (W=32 windows of size 64)
  p = softmax(|a| / tau), tau=1  (over w)
  g[n,h] = sum_w p[n,h,w] * a[n,h,w]

Strategy: 8-way data-parallel over N=4096 tokens (512/core). Host converts
x/W/T to fp16 and pre-transposes so every matmul operand has its contraction
dim on partitions. On-chip per core:
  - einsum: 32 window matmuls [K=64, M=128 tok, N=512 h] -> PSUM fp32
  - gate:  ACT copies a to fp16, DVE abs (4x), ACT exp, DVE mul (2x),
           in-place pairwise trees over the 32 windows for num/den,
           fast-reciprocal, g = num/den
  - linear: 16 accumulating matmuls [K=128, M=128, N=512]
  - finish: silu(g) * lin  (one ACT table switch at the very end)
"""

import sys

if "/opt/trn_rl_repo" not in sys.path:
    sys.path.insert(0, "/opt/trn_rl_repo")

import numpy as np

import concourse.bass as bass
import concourse.tile as tile
from concourse import bacc, mybir
from concourse.bass_utils import run_bass_kernel_spmd

# Problem shapes (hardcoded per harness contract)
N_TOKENS = 4096
D = 2048          # in_features
H = 1024          # out_features
WIN = 64          # window size
NW = 32           # num windows
NCORES = 8
TOK = N_TOKENS // NCORES    # tokens per core = 512
NTT = TOK // 128            # token tiles per core = 4
NHC = H // 512              # h chunks = 2
HALF = 16                   # windows per half
KC = D // 128               # k chunks for linear = 16

F16 = mybir.dt.float16
F32 = mybir.dt.float32


def _build_module():
    nc = bacc.Bacc("TRN2", target_bir_lowering=False, debug=False,
                   num_devices=NCORES)

    xT = nc.dram_tensor("xT", [D, TOK], F16, kind="ExternalInput").ap()
    wT = nc.dram_tensor("wT", [D, H], F16, kind="ExternalInput").ap()
    tT = nc.dram_tensor("tT", [D, H], F16, kind="ExternalInput").ap()
    out = nc.dram_tensor("out", [TOK, H], F32, kind="ExternalOutput").ap()

    with tile.TileContext(nc) as tc, nc.allow_low_precision(
        reason="fp16 gate pipeline by design"
    ):
        _body(tc, nc, xT, wT, tT, out)

    nc.compile()
    return nc


def _body(tc, nc, xT, wT, tT, out):
    from contextlib import ExitStack

    ctx = ExitStack()
    with ctx:
        weights = ctx.enter_context(tc.tile_pool(name="weights", bufs=1))
        persist = ctx.enter_context(tc.tile_pool(name="persist", bufs=1))
        abuf_p = ctx.enter_context(tc.tile_pool(name="abuf", bufs=3))
        ebuf_p = ctx.enter_context(tc.tile_pool(name="ebuf", bufs=2))
        smalls = ctx.enter_context(tc.tile_pool(name="smalls", bufs=2))
        outs_p = ctx.enter_context(tc.tile_pool(name="outs", bufs=2))
        ppool = ctx.enter_context(tc.tile_pool(name="apsum", bufs=4, space="PSUM"))

        # ---- resident weights/activations (fp16, pre-transposed on host) ----
        xT_t, wT_t, tT_t = [], [], []
        for c in range(KC):
            xt = weights.tile([128, TOK], F16, name=f"xT{c}", tag=f"xT{c}")
            nc.sync.dma_start(out=xt[:], in_=xT[c * 128:(c + 1) * 128, :])
            xT_t.append(xt)
            wt = weights.tile([128, H], F16, name=f"wT{c}", tag=f"wT{c}")
            nc.sync.dma_start(out=wt[:], in_=wT[c * 128:(c + 1) * 128, :])
            wT_t.append(wt)
            tt_ = weights.tile([128, H], F16, name=f"tT{c}", tag=f"tT{c}")
            nc.sync.dma_start(out=tt_[:], in_=tT[c * 128:(c + 1) * 128, :])
            tT_t.append(tt_)

        # persistent per-token-tile results
        g_all = [persist.tile([128, NHC, 512], F16, name=f"g{t}", tag=f"g{t}")
                 for t in range(NTT)]
        lin_all = [persist.tile([128, NHC, 512], F16, name=f"lin{t}", tag=f"lin{t}")
                   for t in range(NTT)]

        for tt in range(NTT):
            tok_sl = bass.ts(tt, 128)
            for hc in range(NHC):
                h_sl = bass.ts(hc, 512)

                # ------- gate einsum + linear, interleaved on PE -------
                # (one lin k-chunk after each einsum pair keeps PE's pair
                # production smooth; a lin burst would starve the ACT
                # copy pipeline at unit start)
                lin_ps = ppool.tile([128, 2, 512], F32, tag="apair")
                halves = []
                for half in range(2):
                    a_bf = abuf_p.tile([128, HALF, 512], F16, tag="a_bf")
                    ebuf = ebuf_p.tile([128, HALF, 512], F16, tag="ebuf")
                    for pr in range(HALF // 2):
                        w0 = half * HALF + pr * 2
                        aps = ppool.tile([128, 2, 512], F32, tag="apair")
                        for i in range(2):
                            w = w0 + i
                            ct, ro = w // 2, (w % 2) * WIN
                            nc.tensor.matmul(
                                aps[:, i, :],
                                lhsT=xT_t[ct][ro:ro + WIN, tok_sl],
                                rhs=tT_t[ct][ro:ro + WIN, h_sl],
                                start=True, stop=True,
                            )
                        k = half * 8 + pr
                        nc.tensor.matmul(
                            lin_ps[:, 0, :],
                            lhsT=xT_t[k][:, tok_sl],
                            rhs=wT_t[k][:, h_sl],
                            start=(k == 0), stop=(k == KC - 1),
                            skip_group_check=True,
                        )
                        # fp32 PSUM -> fp16 SBUF on ACT (frees the psum
                        # pair; DVE is the bottleneck engine, keep it off)
                        nc.scalar.copy(
                            out=a_bf[:, pr * 2:pr * 2 + 2, :],
                            in_=aps[:, :, :],
                        )
                        # |a| -> ebuf per quarter: clear fp16 sign bit
                        # (DVE int16 4x mode, batched to amortize overhead)
                        if pr % 4 == 3:
                            qs = slice(pr * 2 - 6, pr * 2 + 2)
                            nc.vector.tensor_scalar(
                                out=ebuf[:, qs, :].bitcast(mybir.dt.uint16),
                                in0=a_bf[:, qs, :].bitcast(mybir.dt.uint16),
                                scalar1=0x7FFF, scalar2=None,
                                op0=mybir.AluOpType.bitwise_and,
                            )
                    # e = exp(|a|) and prod = a*e, quarter-granular so the
                    # DVE mul overlaps the ACT exp of the next quarter
                    for q in range(2):
                        qs = slice(q * 8, (q + 1) * 8)
                        nc.scalar.activation(
                            out=ebuf[:, qs, :], in_=ebuf[:, qs, :],
                            func=mybir.ActivationFunctionType.Exp,
                        )
                        nc.vector.tensor_tensor(
                            out=a_bf[:, qs, :], in0=a_bf[:, qs, :],
                            in1=ebuf[:, qs, :], op=mybir.AluOpType.mult,
                        )
                    # pairwise trees over windows: num in a_bf, den in ebuf
                    # (both DVE: GPSIMD shares an SBUF port with DVE and
                    # measured 3-4x slower - offloading there hurts).
                    # Last level lands in small tiles so the big buffers
                    # free early and the next unit's copies can start.
                    n = HALF // 2
                    while n >= 2:
                        nc.vector.tensor_tensor(
                            out=a_bf[:, 0:n, :], in0=a_bf[:, 0:n, :],
                            in1=a_bf[:, n:2 * n, :], op=mybir.AluOpType.add,
                        )
                        nc.vector.tensor_tensor(
                            out=ebuf[:, 0:n, :], in0=ebuf[:, 0:n, :],
                            in1=ebuf[:, n:2 * n, :], op=mybir.AluOpType.add,
                        )
                        n //= 2
                    nd_h = smalls.tile([128, 2, 512], F16, tag=f"ndh{half}",
                                       name=f"ndh{half}")
                    nc.vector.tensor_tensor(
                        out=nd_h[:, 0, :], in0=a_bf[:, 0, :], in1=a_bf[:, 1, :],
                        op=mybir.AluOpType.add)
                    nc.vector.tensor_tensor(
                        out=nd_h[:, 1, :], in0=ebuf[:, 0, :], in1=ebuf[:, 1, :],
                        op=mybir.AluOpType.add)
                    halves.append(nd_h)

                # merge halves (single fused op), then g = num/den
                nd = halves[0]
                nc.vector.tensor_tensor(
                    out=nd[:], in0=nd[:], in1=halves[1][:],
                    op=mybir.AluOpType.add)
                rcp = smalls.tile([128, 512], F16, tag="rcp")
                from concourse.dve_ops import (
                    RECIPROCAL_APPROX_FAST, RECIP_APPROX_FAST_CONSTS)
                nc.vector._custom_dve(
                    RECIPROCAL_APPROX_FAST, out=rcp[:], in0=nd[:, 1, :],
                    **RECIP_APPROX_FAST_CONSTS)
                g_sl = g_all[tt][:, hc, :]
                nc.vector.tensor_tensor(
                    out=g_sl, in0=nd[:, 0, :], in1=rcp[:],
                    op=mybir.AluOpType.mult)

                # lin PSUM -> SBUF (must precede the finish reads below)
                nc.scalar.copy(out=lin_all[tt][:, hc, :], in_=lin_ps[:, 0, :])

                # ---- finish this (tt, hc): out = silu(g)*lin via tanh ----
                # silu(g) = g*(1+tanh(g/2))/2; tanh shares the ACT table
                # set with Exp, so no table switching anywhere.
                th = smalls.tile([128, 512], F16, tag="th")
                nc.scalar.activation(
                    out=th[:], in_=g_sl,
                    func=mybir.ActivationFunctionType.Tanh, scale=0.5,
                )
                nc.vector.tensor_scalar_mul(out=th[:], in0=th[:], scalar1=0.5)
                nc.vector.tensor_scalar_add(out=th[:], in0=th[:], scalar1=0.5)
                gl = smalls.tile([128, 512], F16, tag="gl")
                nc.vector.tensor_tensor(
                    out=gl[:], in0=g_sl,
                    in1=lin_all[tt][:, hc, :], op=mybir.AluOpType.mult)
                o = outs_p.tile([128, 512], F32, tag="o")
                nc.vector.tensor_tensor(
                    out=o[:], in0=gl[:], in1=th[:],
                    op=mybir.AluOpType.mult)
                nc.sync.dma_start(
                    out=out[bass.ts(tt, 128), bass.ts(hc, 512)], in_=o[:])



_NC_CACHE = None


def _get_module():
    global _NC_CACHE
    if _NC_CACHE is None:
        _NC_CACHE = _build_module()
    return _NC_CACHE


def kernel(x: np.ndarray, template_flat: np.ndarray,
           weights: np.ndarray) -> np.ndarray:
    nc = _get_module()

    xT = np.ascontiguousarray(x.T.astype(np.float16))           # [D, N]
    wT = np.ascontiguousarray(weights.T.astype(np.float16))     # [D, H]
    tT = np.ascontiguousarray(template_flat.T.astype(np.float16))

    in_maps = []
    for c in range(NCORES):
        in_maps.append({
            "xT": np.ascontiguousarray(xT[:, c * TOK:(c + 1) * TOK]),
            "wT": wT,
            "tT": tT,
        })
    res = run_bass_kernel_spmd(nc, in_maps, core_ids=list(range(NCORES)))
    return np.concatenate([res.results[c]["out"] for c in range(NCORES)],
                          axis=0).astype(np.float32)


# revision 21
# speedup vs baseline: 1.0088x; 1.0088x over previous
"""Trainium2 Bass kernel for nn_DendriticLayerSiLU_Template.

out = silu(g) * (x @ W.T), where per (token n, unit h):
  a[n,h,w] = sum_s x[n, w*64+s] * T[h, w*64+s]      (W=32 windows of size 64)
  p = softmax(|a| / tau), tau=1  (over w)
  g[n,h] = sum_w p[n,h,w] * a[n,h,w]

Strategy: 8-way data-parallel over N=4096 tokens (512/core). Host converts
x/W/T to fp16 and pre-transposes so every matmul operand has its contraction
dim on partitions. On-chip per core, per (token-tile, h-chunk) unit:
  - einsum: 32 window matmuls [K=64, M=128 tok, N=512 h] -> PSUM fp32,
    with the 16 linear k-chunk matmuls interleaved to keep PE smooth
  - gate:  ACT copies PSUM pairs to fp16, DVE abs (int16 mask, 4x),
    ACT exp (same table set as tanh -> no table switches), DVE mul (2x),
    in-place pairwise reduction trees over windows for num/den,
    fast-reciprocal, g = num/den
  - finish: out = g*lin*(1+tanh(g/2))/2  (= silu(g)*lin), DMA out
"""

import sys

if "/opt/trn_rl_repo" not in sys.path:
    sys.path.insert(0, "/opt/trn_rl_repo")

import numpy as np

import concourse.bass as bass
import concourse.tile as tile
from concourse import bacc, mybir
from concourse.bass_utils import run_bass_kernel_spmd

# Problem shapes (hardcoded per harness contract)
N_TOKENS = 4096
D = 2048          # in_features
H = 1024          # out_features
WIN = 64          # window size
NW = 32           # num windows
NCORES = 8
TOK = N_TOKENS // NCORES    # tokens per core = 512
NTT = TOK // 128            # token tiles per core = 4
NHC = H // 512              # h chunks = 2
HALF = 16                   # windows per half
KC = D // 128               # k chunks for linear = 16

F16 = mybir.dt.float16
F32 = mybir.dt.float32


def _build_module():
    nc = bacc.Bacc("TRN2", target_bir_lowering=False, debug=False,
                   num_devices=NCORES)

    xT = nc.dram_tensor("xT", [D, TOK], F16, kind="ExternalInput").ap()
    wT = nc.dram_tensor("wT", [D, H], F16, kind="ExternalInput").ap()
    tT = nc.dram_tensor("tT", [D, H], F16, kind="ExternalInput").ap()
    out = nc.dram_tensor("out", [TOK, H], F32, kind="ExternalOutput").ap()

    with tile.TileContext(nc) as tc, nc.allow_low_precision(
        reason="fp16 gate pipeline by design"
    ):
        _body(tc, nc, xT, wT, tT, out)

    nc.compile()
    return nc


def _body(tc, nc, xT, wT, tT, out):
    from contextlib import ExitStack

    from concourse.dve_ops import (
        RECIPROCAL_APPROX_FAST, RECIP_APPROX_FAST_CONSTS)

    ctx = ExitStack()
    with ctx:
        weights = ctx.enter_context(tc.tile_pool(name="weights", bufs=1))
        abuf_p = ctx.enter_context(tc.tile_pool(name="abuf", bufs=3))
        ebuf_p = ctx.enter_context(tc.tile_pool(name="ebuf", bufs=3))
        smalls = ctx.enter_context(tc.tile_pool(name="smalls", bufs=2))
        outs_p = ctx.enter_context(tc.tile_pool(name="outs", bufs=2))
        ppool = ctx.enter_context(tc.tile_pool(name="apsum", bufs=4, space="PSUM"))

        # ---- resident weights/activations (fp16, pre-transposed on host) ----
        xT_t, wT_t, tT_t = [], [], []
        for c in range(KC):
            xt = weights.tile([128, TOK], F16, name=f"xT{c}", tag=f"xT{c}")
            nc.sync.dma_start(out=xt[:], in_=xT[c * 128:(c + 1) * 128, :])
            xT_t.append(xt)
            wt = weights.tile([128, H], F16, name=f"wT{c}", tag=f"wT{c}")
            nc.sync.dma_start(out=wt[:], in_=wT[c * 128:(c + 1) * 128, :])
            wT_t.append(wt)
            tt_ = weights.tile([128, H], F16, name=f"tT{c}", tag=f"tT{c}")
            nc.sync.dma_start(out=tt_[:], in_=tT[c * 128:(c + 1) * 128, :])
            tT_t.append(tt_)

        for tt in range(NTT):
            tok_sl = bass.ts(tt, 128)
            for hc in range(NHC):
                h_sl = bass.ts(hc, 512)

                # ------- gate einsum + linear, interleaved on PE -------
                # (one lin k-chunk after each einsum pair keeps PE's pair
                # production smooth; a lin burst would starve the ACT
                # copy pipeline at unit start)
                lin_ps = ppool.tile([128, 2, 512], F32, tag="apair")
                halves = []
                for half in range(2):
                    a_bf = abuf_p.tile([128, HALF, 512], F16, tag="a_bf")
                    ebuf = ebuf_p.tile([128, HALF, 512], F16, tag="ebuf")
                    for pr in range(HALF // 2):
                        w0 = half * HALF + pr * 2
                        aps = ppool.tile([128, 2, 512], F32, tag="apair")
                        for i in range(2):
                            w = w0 + i
                            ct, ro = w // 2, (w % 2) * WIN
                            nc.tensor.matmul(
                                aps[:, i, :],
                                lhsT=xT_t[ct][ro:ro + WIN, tok_sl],
                                rhs=tT_t[ct][ro:ro + WIN, h_sl],
                                start=True, stop=True,
                            )
                        k = half * 8 + pr
                        nc.tensor.matmul(
                            lin_ps[:, 0, :],
                            lhsT=xT_t[k][:, tok_sl],
                            rhs=wT_t[k][:, h_sl],
                            start=(k == 0), stop=(k == KC - 1),
                            skip_group_check=True,
                        )
                        # fp32 PSUM -> fp16 SBUF on ACT (frees the psum
                        # pair; DVE is the bottleneck engine, keep it off)
                        nc.scalar.copy(
                            out=a_bf[:, pr * 2:pr * 2 + 2, :],
                            in_=aps[:, :, :],
                        )
                        # |a| -> ebuf per quarter: clear fp16 sign bit
                        # (DVE int16 4x mode, batched to amortize overhead)
                        if pr % 4 == 3:
                            qs = slice(pr * 2 - 6, pr * 2 + 2)
                            nc.vector.tensor_scalar(
                                out=ebuf[:, qs, :].bitcast(mybir.dt.uint16),
                                in0=a_bf[:, qs, :].bitcast(mybir.dt.uint16),
                                scalar1=0x7FFF, scalar2=None,
                                op0=mybir.AluOpType.bitwise_and,
                            )
                    # e = exp(|a|) and prod = a*e, quarter-granular so the
                    # DVE mul overlaps the ACT exp of the next quarter
                    for q in range(2):
                        qs = slice(q * 8, (q + 1) * 8)
                        nc.scalar.activation(
                            out=ebuf[:, qs, :], in_=ebuf[:, qs, :],
                            func=mybir.ActivationFunctionType.Exp,
                        )
                        nc.vector.tensor_tensor(
                            out=a_bf[:, qs, :], in0=a_bf[:, qs, :],
                            in1=ebuf[:, qs, :], op=mybir.AluOpType.mult,
                        )
                    # pairwise trees over windows: num in a_bf, den in ebuf
                    # (both DVE: GPSIMD shares an SBUF port with DVE and
                    # measured 3-4x slower - offloading there hurts).
                    # Last level lands in a small combined [num|den] tile
                    # so the big buffers free early.
                    n = HALF // 2
                    while n >= 2:
                        nc.vector.tensor_tensor(
                            out=a_bf[:, 0:n, :], in0=a_bf[:, 0:n, :],
                            in1=a_bf[:, n:2 * n, :], op=mybir.AluOpType.add,
                        )
                        nc.vector.tensor_tensor(
                            out=ebuf[:, 0:n, :], in0=ebuf[:, 0:n, :],
                            in1=ebuf[:, n:2 * n, :], op=mybir.AluOpType.add,
                        )
                        n //= 2
                    nd_h = smalls.tile([128, 2, 512], F16, tag=f"ndh{half}",
                                       name=f"ndh{half}")
                    nc.vector.tensor_tensor(
                        out=nd_h[:, 0, :], in0=a_bf[:, 0, :], in1=a_bf[:, 1, :],
                        op=mybir.AluOpType.add)
                    nc.vector.tensor_tensor(
                        out=nd_h[:, 1, :], in0=ebuf[:, 0, :], in1=ebuf[:, 1, :],
                        op=mybir.AluOpType.add)
                    halves.append(nd_h)

                # merge halves (single fused op), then g = num/den
                nd = halves[0]
                nc.vector.tensor_tensor(
                    out=nd[:], in0=nd[:], in1=halves[1][:],
                    op=mybir.AluOpType.add)
                rcp = smalls.tile([128, 512], F16, tag="rcp")
                nc.vector._custom_dve(
                    RECIPROCAL_APPROX_FAST, out=rcp[:], in0=nd[:, 1, :],
                    **RECIP_APPROX_FAST_CONSTS)
                g = smalls.tile([128, 512], F16, tag="g")
                nc.vector.tensor_tensor(
                    out=g[:], in0=nd[:, 0, :], in1=rcp[:],
                    op=mybir.AluOpType.mult)

                # lin PSUM -> fp16 SBUF (must precede the finish reads)
                lin = smalls.tile([128, 512], F16, tag="lin")
                nc.scalar.copy(out=lin[:], in_=lin_ps[:, 0, :])

                # ---- finish: out = g*lin*(1+tanh(g/2))/2 = silu(g)*lin ----
                # tanh shares the ACT table set with Exp: no table switches.
                # Recycle dead tiles (rcp, nd) to save SBUF.
                th = rcp
                nc.scalar.activation(
                    out=th[:], in_=g[:],
                    func=mybir.ActivationFunctionType.Tanh, scale=0.5,
                )
                gl = nd[:, 0, :]
                nc.vector.scalar_tensor_tensor(
                    out=gl, in0=g[:], scalar=0.5, in1=lin[:],
                    op0=mybir.AluOpType.mult, op1=mybir.AluOpType.mult)
                o = outs_p.tile([128, 512], F32, tag="o")
                nc.vector.scalar_tensor_tensor(
                    out=o[:], in0=th[:], scalar=1.0, in1=gl,
                    op0=mybir.AluOpType.add, op1=mybir.AluOpType.mult)
                nc.sync.dma_start(
                    out=out[bass.ts(tt, 128), bass.ts(hc, 512)], in_=o[:])


_NC_CACHE = None


def _get_module():
    global _NC_CACHE
    if _NC_CACHE is None:
        _NC_CACHE = _build_module()
    return _NC_CACHE


def kernel(x: np.ndarray, template_flat: np.ndarray,
           weights: np.ndarray) -> np.ndarray:
    nc = _get_module()

    xT = np.ascontiguousarray(x.T.astype(np.float16))           # [D, N]
    wT = np.ascontiguousarray(weights.T.astype(np.float16))     # [D, H]
    tT = np.ascontiguousarray(template_flat.T.astype(np.float16))

    in_maps = []
    for c in range(NCORES):
        in_maps.append({
            "xT": np.ascontiguousarray(xT[:, c * TOK:(c + 1) * TOK]),
            "wT": wT,
            "tT": tT,
        })
    res = run_bass_kernel_spmd(nc, in_maps, core_ids=list(range(NCORES)))
    return np.concatenate([res.results[c]["out"] for c in range(NCORES)],
                          axis=0).astype(np.float32)
